# revision 86
# baseline (speedup 1.0000x reference)
"""Trainium2 Bass kernel for nn_GCNTime (GCN + per-t causal transformer over nodes).

Sharding: T=16 time steps across 8 cores (2 per core). The graph (dense
normalized adjacency) is replicated; every stage is independent across t,
so there are no collectives.

v4 layout notes (per core):
- Activations are feature-major [128 partitions, token] (token = t*2048+node).
- PSUM is managed as three tag pools: "big"/"st" [128,1024] (two banks) and
  "pA"/"pB" [128,512]; paired 1024-wide tiles halve evac/exp instruction
  counts.
- Attention scores are computed transposed (keys on partitions); the causal
  block-triangle is trimmed at 128-column granularity and only the single
  diagonal 128x128 sub-block per key-block needs a mask multiply.
- Softmax denominator and LN statistics are partition reductions via
  ones-matmuls.
- LayerNorm rstd = exp(-0.5*ln(P^2*var + P^2*eps) + ln P): Ln/Exp share the
  one Act LUT set (natural_log_exp_and_others, id 6) with attention's Exp
  and all Relu/Square/Identity/Copy ops; the set is pre-loaded once so the
  Act engine never reloads activation tables mid-kernel.
- Layer-1 GCN aggregation runs as two half-sweeps over the resident s
  tiles with the psum evac / conv / qkv chains interleaved into the second
  half, hiding those chains behind PE matmul streams.
- FFN f1 evacs alternate Act/DVE evenly (4/4) and the f1 ring is 4 deep so
  the w2 accumulation never waits on a relu evac.
- Cross-boundary tail deferral: each attention chunk's last two dnav
  groups are re-emitted behind the NEXT chunk's first score pair, and each
  FFN step's last two w2 groups + z2 residual ride behind the next step's
  first w1 pairs, so the PE never stalls waiting for a boundary's final
  exp/relu evacuations (in-order queues would otherwise serialize them).
- Layer0 -> layer1 node-major input is produced by DMA XBAR transposes
  (bf16), not PE transposes. The final output stays feature-major and is
  transposed on the host.
- Weights arrive as two packed DMAs (bf16 + f32); only the conv/qkv/wo
  columns + attention masks ride the startup-critical s_t stream -- the
  1MB of FFN weight columns load after the layer-0 aggregation on the
  then-idle scalar DMA queue.
- Attention epilogues (softmax reciprocal -> wo matmul -> z1 residual)
  are deferred two chunks so their cross-engine chains never block the
  next chunks' score matmuls.
- Engine placement tuned end-to-end: post-relu +pe on DVE, agg psum
  evacs split DVE/Act to match consumer timing, FFN relu evacs 4/4
  Act/DVE, LN sq/varq tile rings 3 deep.
"""

import math
from contextlib import ExitStack

import numpy as np
import ml_dtypes

import concourse.bacc as bacc
import concourse.tile as tile
from concourse import mybir
from concourse.bass_utils import run_bass_kernel_spmd

P = 128
N = 2048          # nodes
T = 16            # total time steps
TL = 2            # time steps per core
NB = N // P       # node blocks (16)
H = 128
DFF = 2048
NF = DFF // P     # ffn chunks (16)
L = 2
TOKS = TL * N     # tokens per core (4096)
NDC = N // 512    # 512-wide node chunks (4)
TC = TOKS // 512  # 512-wide token chunks (8)
EPS = 1e-5
SCALE = 1.0 / math.sqrt(H)
SQH = math.sqrt(H)

f32 = mybir.dt.float32
f32r = mybir.dt.float32r
bf16 = mybir.dt.bfloat16
bfnp = ml_dtypes.bfloat16

AF = mybir.ActivationFunctionType
ALU = mybir.AluOpType

# ---- packed weight column offsets (bf16 pack) ----
# per layer: convW H | wqkvT 3H | woT H | w1T DFF | w2T NF*H
_LW = H + 3 * H + H + DFF + NF * H          # 4736
BCOLS = L * _LW + 4 * P                      # + ones_b + negI + slt + negrow
NEG_BIG = float(2 ** 20)
# f32 pack per layer: convb_s 1 | bq 1 | bk 1 | bo 1 | b1 NF | b2 1 |
#                     ln1g 1 | ln1b 1 | ln2g 1 | ln2b 1
_LF = 8 + NF
FCOLS = L * _LF + 3                          # + eps, P^2*eps, ln(P)


def _boffs(l):
    o = l * _LW
    return {
        "convW": o, "wqkvT": o + H, "woT": o + 4 * H,
        "w1T": o + 5 * H, "w2T": o + 5 * H + DFF,
    }


def _foffs(l):
    o = l * _LF
    return {
        "convb_s": o, "bq": o + 1, "bk": o + 2, "bo": o + 3,
        "b1": o + 4, "b2": o + 4 + NF, "ln1g": o + 5 + NF,
        "ln1b": o + 6 + NF, "ln2g": o + 7 + NF, "ln2b": o + 8 + NF,
    }


def _emit(tc, io, lean):
    nc = tc.nc
    with ExitStack() as ctx:
        consts = ctx.enter_context(tc.tile_pool(name="consts", bufs=1))
        spool = ctx.enter_context(tc.tile_pool(name="spool", bufs=14))
        xpool = ctx.enter_context(tc.tile_pool(name="xpool", bufs=1))
        resid = ctx.enter_context(tc.tile_pool(name="resid", bufs=2))
        hbp = ctx.enter_context(tc.tile_pool(name="hbp", bufs=2))
        qkvp = ctx.enter_context(tc.tile_pool(name="qkvp", bufs=2))
        apool = ctx.enter_context(tc.tile_pool(name="apool", bufs=3))
        ffp = ctx.enter_context(tc.tile_pool(name="ffp", bufs=4))
        small = ctx.enter_context(tc.tile_pool(name="small", bufs=2))
        psum = ctx.enter_context(tc.tile_pool(name="psum", bufs=2, space="PSUM"))

        nc.scalar.add_instruction(mybir.InstLoadActFuncSet(
            name=nc.get_next_instruction_name(), act_func_set_id=6))

        sdma = nc.sync.dma_start
        adma = nc.scalar.dma_start
        wdma = nc.gpsimd.dma_start

        # ---- initial DMAs: critical data first (xnm t-halves + s_t c=0,1),
        # then packed weights on the software-DGE queue ----
        xnm = xpool.tile([P, TL, NB, P], bf16, tag="xnm", name="xnm0")
        sdma(out=xnm[:, 0, 0:4], in_=io["x_nm"][:, 0, 0:4])
        s_pre = []
        s0 = spool.tile([P, N], bf16, tag="s_tile", name="s0_0")
        adma(out=s0[:, 0:1024], in_=io["s_t"][0][:, 0:1024])
        sdma(out=xnm[:, 0, 4:NB], in_=io["x_nm"][:, 0, 4:NB])
        adma(out=s0[:, 1024:2048], in_=io["s_t"][0][:, 1024:2048])
        s_pre.append(s0)
        s1 = spool.tile([P, N], bf16, tag="s_tile", name="s0_1")
        adma(out=s1, in_=io["s_t"][1])
        sdma(out=xnm[:, 1], in_=io["x_nm"][:, 1])
        s_pre.append(s1)

        # wb pieces are DMA'd interleaved with the layer-0 s_t stream (below)
        # so the big weight transfers don't block the startup-critical loads
        wb = consts.tile([P, BCOLS], bf16, tag="wb")
        # layer-0 pieces + masks stream during the startup-critical window;
        # layer-1 pieces wait until the DMA engines go idle (conv/qkv phase)
        wb_pieces = [
            (0, 5 * H), (L * _LW, 4 * P),
        ]
        wb_late = [
            (5 * H, DFF), (5 * H + DFF, NF * H),
            (_LW, 5 * H), (_LW + 5 * H, DFF), (_LW + 5 * H + DFF, NF * H),
        ]
        wf = consts.tile([P, FCOLS], f32, tag="wf")
        wdma(out=wf, in_=io["wf"])
        pe_t = consts.tile([P, TL], f32, tag="pe_t")
        wdma(out=pe_t, in_=io["pe_t"])
        ones_f = consts.tile([P, P], f32r, tag="ones_f")
        wdma(out=ones_f, in_=io["ones_f"])
        bvr = consts.tile([1, L * H], bf16, tag="bvr")
        wdma(out=bvr, in_=io["bvr"])

        ones_b = wb[:, L * _LW:L * _LW + P]
        negI = wb[:, L * _LW + P:L * _LW + 2 * P]
        slt = wb[:, L * _LW + 2 * P:L * _LW + 3 * P]
        negrow = wb[:, L * _LW + 3 * P:L * _LW + 4 * P]
        eps_t = (wf[:, L * _LF + 1:L * _LF + 2], wf[:, L * _LF + 2:L * _LF + 3])

        s_next = None
        for l in range(L):
            bo_ = _boffs(l)
            fo = _foffs(l)
            convW = wb[:, bo_["convW"]:bo_["convW"] + H]
            wqkvT = wb[:, bo_["wqkvT"]:bo_["wqkvT"] + 3 * H]
            woT = wb[:, bo_["woT"]:bo_["woT"] + H]
            w1T = wb[:, bo_["w1T"]:bo_["w1T"] + DFF]
            w2T = wb[:, bo_["w2T"]:bo_["w2T"] + NF * H]
            convb_s = wf[:, fo["convb_s"]:fo["convb_s"] + 1]
            bq = wf[:, fo["bq"]:fo["bq"] + 1]
            bk = wf[:, fo["bk"]:fo["bk"] + 1]
            bo_ap = wf[:, fo["bo"]:fo["bo"] + 1]
            b1 = wf[:, fo["b1"]:fo["b1"] + NF]
            b2 = wf[:, fo["b2"]:fo["b2"] + 1]
            ln1g = wf[:, fo["ln1g"]:fo["ln1g"] + 1]
            ln1b = wf[:, fo["ln1b"]:fo["ln1b"] + 1]
            ln2g = wf[:, fo["ln2g"]:fo["ln2g"] + 1]
            ln2b = wf[:, fo["ln2b"]:fo["ln2b"] + 1]

            # ======== GCN aggregation: agg[f, tok] = sum_s x[s,f] * S^T[s, dst]
            # 5 accumulator psums covering [t, dc]: big(t0,dc01) big(t0,dc23)
            # st(t1,dc01) ps(t1,dc2) ps(t1,dc3)
            agA = psum.tile([P, 1024], f32, tag="big", name=f"agA{l}")
            agB = psum.tile([P, 1024], f32, tag="big", name=f"agB{l}")
            agC = psum.tile([P, 1024], f32, tag="st", bufs=1, name=f"agC{l}")
            agD = psum.tile([P, 512], f32, tag="pA", bufs=1, name=f"agD{l}")
            agE = psum.tile([P, 512], f32, tag="pB", bufs=1, name=f"agE{l}")
            dsts = [
                (0, agA[:, 0:512]), (0, agA[:, 512:1024]),
                (0, agB[:, 0:512]), (0, agB[:, 512:1024]),
                (1, agC[:, 0:512]), (1, agC[:, 512:1024]),
                (1, agD), (1, agE),
            ]
            aggb = (hbp.tile([P, TOKS], bf16, tag="hb", name=f"aggb{l}")
                    if l > 0 else None)
            h1b = hbp.tile([P, TOKS], bf16, tag="hb", name=f"h1b{l}")
            evacs = [
                lambda: nc.vector.tensor_copy(aggb[:, 0:1024], agA),
                lambda: nc.scalar.copy(aggb[:, 1024:2048], agB),
                lambda: nc.vector.tensor_copy(aggb[:, 2048:3072], agC),
                lambda: nc.scalar.copy(aggb[:, 3072:3584], agD),
                lambda: nc.vector.tensor_copy(aggb[:, 3584:4096], agE),
            ]

            def agg_sweep(idxs, c0, c1):
                for c in range(c0, c1):
                    s_tile = s_next[c]
                    for i in idxs:
                        t, dst = dsts[i]
                        dc = i % 4
                        nc.tensor.matmul(
                            dst, xnm[:, t, c],
                            s_tile[:, dc * 512:(dc + 1) * 512],
                            start=(c == 0), stop=(c == NB - 1))

            # GCN linear -> relu*sqrt(H) -> +pe -> h1b (bf16 residual)
            def emit_conv(pr):
                t = (2 * pr) // NDC
                pc = psum.tile([P, 1024], f32, tag="big", name=f"conv{l}_{pr}")
                sl = slice(pr * 1024, (pr + 1) * 1024)
                nc.tensor.matmul(pc[:, 0:512], convW,
                                 aggb[:, pr * 1024:pr * 1024 + 512],
                                 start=True, stop=True)
                nc.tensor.matmul(pc[:, 512:1024], convW,
                                 aggb[:, pr * 1024 + 512:(pr + 1) * 1024],
                                 start=True, stop=True)
                nc.scalar.activation(out=h1b[:, sl], in_=pc, func=AF.Relu,
                                     scale=SQH, bias=convb_s)
                nc.vector.tensor_scalar(
                    out=h1b[:, sl], in0=h1b[:, sl], scalar1=pe_t[:, t:t + 1],
                    scalar2=None, op0=ALU.add)

            # qkv for one t (q,k feature-major; v node-major)
            qs, ks, vns, attns = [], [], [], []

            def emit_qkv(t):
                q_t = qkvp.tile([P, N], bf16, tag="q_t", name=f"q{l}_{t}")
                k_t = qkvp.tile([P, N], bf16, tag="k_t", name=f"k{l}_{t}")
                for part, dest, bias in ((0, q_t, bq), (1, k_t, bk)):
                    for hp in range(2):
                        pq = psum.tile([P, 1024], f32, tag="big",
                                       name=f"qk{l}_{t}_{part}_{hp}")
                        for hh in range(2):
                            ncc = hp * 2 + hh
                            nc.tensor.matmul(
                                pq[:, hh * 512:(hh + 1) * 512],
                                wqkvT[:, part * H:(part + 1) * H],
                                h1b[:, t * N + ncc * 512:t * N + (ncc + 1) * 512],
                                start=True, stop=True)
                        osl = dest[:, hp * 1024:(hp + 1) * 1024]
                        if part == 0:
                            nc.vector.tensor_scalar(
                                out=osl, in0=pq, scalar1=bias,
                                scalar2=None, op0=ALU.add)
                        else:
                            nc.scalar.activation(
                                out=osl, in_=pq, func=AF.Identity, bias=bias)
                vn = qkvp.tile([P, NB, P], bf16, tag="vn", name=f"vn{l}_{t}")
                for hp in range(2):
                    pv = psum.tile([P, 1024], f32, tag="big",
                                   name=f"v{l}_{t}_{hp}")
                    for j in range(8):
                        jj = hp * 8 + j
                        reg = pv[:, j * P:(j + 1) * P]
                        nc.tensor.matmul(
                            reg,
                            h1b[:, t * N + jj * P:t * N + (jj + 1) * P],
                            wqkvT[:, 2 * H:3 * H],
                            start=True, stop=lean)
                        if not lean:
                            # bias via rank-1 accumulation (ones row x bias row)
                            nc.tensor.matmul(
                                reg, ones_b[0:1, :],
                                bvr[:, l * H:(l + 1) * H],
                                start=False, stop=True)
                    nc.vector.tensor_copy(
                        vn[:, hp * 8:(hp + 1) * 8, :], pv)
                attnb = qkvp.tile([P, N], bf16, tag="attnb", name=f"at{l}_{t}")
                qs.append(q_t); ks.append(k_t); vns.append(vn); attns.append(attnb)

            if l == 0:
                # layer 0: s_t streams from HBM; tile-major sweep keeps the
                # PE fed at the DMA arrival rate
                for c in range(NB):
                    if c < 2:
                        s_tile = s_pre[c]
                    else:
                        s_tile = spool.tile([P, N], bf16, tag="s_tile",
                                            name=f"s{l}_{c}")
                        (sdma if c % 2 == 0 else adma)(
                            out=s_tile, in_=io["s_t"][c])
                    if 1 <= c <= len(wb_pieces):
                        off, w = wb_pieces[c - 1]
                        (sdma if c % 2 == 1 else adma)(
                            out=wb[:, off:off + w], in_=io["wb"][:, off:off + w])
                    for i, (t, dst) in enumerate(dsts):
                        dc = i % 4
                        nc.tensor.matmul(
                            dst, xnm[:, t, c], s_tile[:, dc * 512:(dc + 1) * 512],
                            start=(c == 0), stop=(c == NB - 1))
                # prefetch next layer's adjacency tiles during this layer's
                # compute (DMA engines are otherwise idle mid-layer)
                s_next = []
                for c in range(NB):
                    st_ = spool.tile([P, N], bf16, tag="s_tile",
                                     name=f"s{l + 1}_{c}")
                    sdma(out=st_, in_=io["s_t"][c])
                    s_next.append(st_)
                for off, w in wb_late:
                    sdma(out=wb[:, off:off + w], in_=io["wb"][:, off:off + w])
                for e in evacs:
                    e()
                for pr in range(TC // 2):
                    emit_conv(pr)
                emit_qkv(0)
                emit_qkv(1)
            else:
                # layer 1: s tiles prefetched. Sweep tiles 0..7 for all
                # psums first (tile release order keeps the 13-deep spool
                # ring from gating the tail prefetch DMAs), then finish each
                # psum over tiles 8..15 with its evac/conv/qkv interleaved so
                # those chains hide behind the next psum's matmuls.
                for idxs in ([0, 1], [2, 3], [4, 5], [6, 7]):
                    agg_sweep(idxs, 0, NB // 2)
                agg_sweep([0, 1], NB // 2, NB)
                evacs[0]()
                emit_conv(0)
                agg_sweep([2, 3], NB // 2, NB)
                evacs[1]()
                emit_conv(1)
                emit_qkv(0)
                agg_sweep([4, 5], NB // 2, NB)
                evacs[2]()
                emit_conv(2)
                agg_sweep([6, 7], NB // 2, NB)
                evacs[3]()
                evacs[4]()
                emit_conv(3)
                emit_qkv(1)

            # ======== attention, (ic, t)-interleaved, causal-trimmed
            z1 = resid.tile([P, TOKS], f32r, tag="resid", name=f"z1_{l}")
            yl1b = hbp.tile([P, TOKS], bf16, tag="hb", name=f"yl1b{l}")
            ci = 0
            pend_ep = []
            pend_dnav = []

            def attn_chunk(ic, t):
                    nonlocal ci
                    q_t, k_t, vn, attnb = qs[t], ks[t], vns[t], attns[t]
                    jmax = 4 * ic + 4
                    npair = jmax // 2
                    q0 = ic * 512

                    def rs(j):
                        return 128 * max(0, j - 4 * ic)

                    # pd/pv double-buffer across chunks: even chunks use the
                    # pA/pB half-banks, odd chunks the st pair
                    if ci % 2 == 0:
                        pd = psum.tile([P, 512], f32, tag="pA", bufs=1,
                                       name=f"pd{l}_{t}_{ic}")
                        pv = psum.tile([P, 512], f32, tag="pB", bufs=1,
                                       name=f"pv{l}_{t}_{ic}")
                    else:
                        pdpv = psum.tile([P, 1024], f32, tag="st", bufs=1,
                                         name=f"pdpv{l}_{t}_{ic}")
                        pd, pv = pdpv[:, 0:512], pdpv[:, 512:1024]
                    pairs = []

                    def emit_dnav(p):
                        A_p, j0 = pairs[p]
                        for h in range(2):
                            j = j0 + h
                            r0 = rs(j)
                            nc.tensor.matmul(
                                pd[:, r0:512], ones_b,
                                A_p[:, h * 512 + r0:(h + 1) * 512],
                                start=(j == 0), stop=(j == jmax - 1))
                        for h in range(2):
                            j = j0 + h
                            r0 = rs(j)
                            nc.tensor.matmul(
                                pv[:, r0:512], vn[:, j, :],
                                A_p[:, h * 512 + r0:(h + 1) * 512],
                                start=(j == 0), stop=(j == jmax - 1))

                    for p in range(npair):
                        j0 = 2 * p
                        pa = psum.tile([P, 1024], f32, tag="big",
                                       name=f"sc{l}_{t}_{ic}_{p}")
                        diag = j0 >= 4 * ic
                        for h in range(2):
                            j = j0 + h
                            r0 = rs(j)
                            nc.tensor.matmul(
                                pa[:, h * 512 + r0:(h + 1) * 512],
                                k_t[:, j * P:(j + 1) * P],
                                q_t[:, q0 + r0:q0 + 512],
                                start=True, stop=not diag,
                                skip_group_check=diag)
                            if diag:
                                # causal mask: add -BIG to the upper triangle
                                # of the diagonal 128x128 sub-block (exp -> 0)
                                nc.tensor.matmul(
                                    pa[:, h * 512 + r0:h * 512 + r0 + 128],
                                    negI, slt, start=False, stop=True,
                                    skip_group_check=True)
                        A_p = apool.tile([P, 1024], bf16, tag="A",
                                         name=f"A{l}_{t}_{ic}_{p}")
                        r0 = rs(j0)
                        nc.scalar.activation(out=A_p[:, r0:1024],
                                             in_=pa[:, r0:1024],
                                             func=AF.Exp, scale=SCALE)
                        pairs.append((A_p, j0))
                        if p == 0 and pend_dnav:
                            # previous chunk's tail dnav groups ride behind
                            # our first scores: their exp inputs are ready
                            # by now, so the PE never stalls on the exp of
                            # a chunk's last pairs
                            for f in pend_dnav:
                                f()
                            pend_dnav.clear()
                        if p >= 2:
                            emit_dnav(p - 2)
                    pend_dnav.append(
                        lambda p=npair - 2, f=emit_dnav: f(p))
                    pend_dnav.append(
                        lambda p=npair - 1, f=emit_dnav: f(p))

                    # defer the chunk epilogue (rec/attnb/po/z1) until the
                    # next chunk's scores are on the PE queue, so the po
                    # matmul's wait on the DVE chain doesn't block them
                    def mk_epilogue(pd=pd, pv=pv, t=t, ic=ic, ci=ci,
                                    attnb=attnb, q0=q0):
                        def ep():
                            rec = small.tile([P, 512], f32, tag="rec", bufs=2,
                                             name=f"rec{l}_{t}_{ic}")
                            nc.vector.reciprocal(rec, pd)
                            nc.vector.tensor_tensor(
                                out=attnb[:, q0:q0 + 512], in0=pv,
                                in1=rec, op=ALU.mult)
                            po = psum.tile([P, 512], f32,
                                           tag=("pA" if ci % 2 == 0 else "pB"),
                                           bufs=1, name=f"wo{l}_{t}_{ic}")
                            nc.tensor.matmul(po, woT, attnb[:, q0:q0 + 512],
                                             start=True, stop=True)
                            sl = slice(t * N + q0, t * N + q0 + 512)
                            nc.vector.scalar_tensor_tensor(
                                out=z1[:, sl], in0=po, scalar=bo_ap,
                                in1=h1b[:, sl], op0=ALU.add, op1=ALU.add)
                        return ep
                    pend_ep.append(mk_epilogue())
                    ci += 1
                    if ci == 4:
                        # hoist the first two LN1 chunks (their z1 tokens are
                        # complete) so the LN chain overlaps late attention
                        for f in pend_dnav:
                            f()
                        pend_dnav.clear()
                        while pend_ep:
                            pend_ep.pop(0)()
                        for e in range(2):
                            self_ln(tc, psum, small, ones_f, eps_t, z1, yl1b,
                                    None, ln1g, ln1b, lean, f"ln1_{l}", e)
                    elif len(pend_ep) > 2:
                        pend_ep.pop(0)()

            # agg/conv/qkv, with attention chunk (0,0) emitted before
            # qkv(t=1) so its scores fill the qkv evac bubbles on the PE
            emit_front()
            for ic in range(NDC):
                for t in range(TL):
                    attn_chunk(ic, t)
            for f in pend_dnav:
                f()
            pend_dnav.clear()
            while pend_ep:
                pend_ep.pop(0)()

            # ======== FFN (+ remaining LN1) -> z2 ; then LN2
            z2 = resid.tile([P, TOKS], f32r, tag="resid", name=f"z2_{l}")

            pend_ffn = []

            def emit_ffn(tch):
                sl = slice(tch * 512, (tch + 1) * 512)
                p2 = psum.tile([P, 512], f32, tag="pA", bufs=1,
                               name=f"ff2{l}_{tch}")
                f1s = []

                def emit_w2(cp):
                    f1 = f1s[cp]
                    for h in range(2):
                        c = 2 * cp + h
                        nc.tensor.matmul(
                            p2, w2T[:, c * P:(c + 1) * P],
                            f1[:, h * 512:(h + 1) * 512],
                            start=(c == 0), stop=(c == NF - 1))

                for cp in range(NF // 2):
                    p1 = psum.tile([P, 1024], f32, tag="big",
                                   name=f"ff1{l}_{tch}_{cp}")
                    for h in range(2):
                        c = 2 * cp + h
                        nc.tensor.matmul(
                            p1[:, h * 512:(h + 1) * 512],
                            w1T[:, c * P:(c + 1) * P], yl1b[:, sl],
                            start=True, stop=True)
                    f1 = ffp.tile([P, 1024], bf16, tag="f1",
                                  name=f"f1_{l}_{tch}_{cp}")
                    eng = (0, 1, 0, 1, 0, 1, 0, 1)[cp % 8]
                    if lean:
                        if eng == 0:
                            nc.scalar.activation(out=f1, in_=p1, func=AF.Relu)
                        else:
                            nc.vector.tensor_scalar(
                                out=f1, in0=p1, scalar1=0.0, scalar2=None,
                                op0=ALU.max)
                    else:
                        for h in range(2):
                            c = 2 * cp + h
                            hs = slice(h * 512, (h + 1) * 512)
                            if eng == 0:
                                nc.scalar.activation(
                                    out=f1[:, hs], in_=p1[:, hs], func=AF.Relu,
                                    bias=b1[:, c:c + 1])
                            else:
                                nc.vector.tensor_scalar(
                                    out=f1[:, hs], in0=p1[:, hs],
                                    scalar1=b1[:, c:c + 1], scalar2=0.0,
                                    op0=ALU.add, op1=ALU.max)
                    f1s.append(f1)
                    if cp == 1 and pend_ffn:
                        # previous step's deferred tail (last two w2 groups
                        # + z2 residual) rides behind our first w1 pairs:
                        # its relu inputs are long done, so the PE never
                        # stalls on a step's last evacs. Flushing at cp==1
                        # keeps the 4-deep f1 ring's eviction waits backward.
                        for f in pend_ffn:
                            f()
                        pend_ffn.clear()
                    if cp >= 2:
                        emit_w2(cp - 2)

                def ffn_tail(emit_w2=emit_w2, p2=p2, sl=sl):
                    emit_w2(NF // 2 - 2)
                    emit_w2(NF // 2 - 1)
                    nc.vector.scalar_tensor_tensor(
                        out=z2[:, sl], in0=p2, scalar=b2, in1=yl1b[:, sl],
                        op0=ALU.add, op1=ALU.add)
                pend_ffn.append(ffn_tail)

            # LN1 / FFN / LN2 software-pipelined per 512-token chunk; LN2
            # feeds the next layer's node-major input (DMA transpose) or the
            # final feature-major output DMA.
            if l < L - 1:
                youtb = hbp.tile([P, TOKS], bf16, tag="hb", name=f"ynb{l}")
                xnm = xpool.tile([P, TL, NB, P], bf16, tag="xnm", name="xnm1")
                yo = None
            else:
                youtb = None

            def emit_ln2(tch):
                if youtb is not None:
                    self_ln(tc, psum, small, ones_f, eps_t, z2, youtb, None,
                            ln2g, ln2b, lean, f"ln2_{l}", tch)
                    t, ic = tch // NDC, tch % NDC
                    nc.sync.dma_start_transpose(
                        xnm[:, t, 4 * ic:4 * ic + 4, :],
                        youtb[:, tch * 512:(tch + 1) * 512])
                else:
                    yo_c = small.tile([P, 512], f32, tag="yoc", bufs=2,
                                      name=f"yoc{tch}")
                    dq = sdma if tch % 2 == 0 else adma
                    if tch == TC - 1:
                        # split the last chunk for a shorter drain chain,
                        # with stats in the (now free) pA/pB banks so the
                        # halves don't serialize on the shared st bank
                        for hh in range(2):
                            pp = psum.tile([P, 512], f32,
                                           tag=("pA" if hh == 0 else "pB"),
                                           bufs=1, name=f"lnd{l}_{hh}")
                            self_ln(tc, psum, small, ones_f, eps_t, z2, None,
                                    yo_c, ln2g, ln2b, lean, f"ln2_{l}_h{hh}",
                                    tch, col0=tch * 512 + hh * 256, W=256,
                                    pst_ext=(pp[:, 0:256], pp[:, 256:512]))
                            sl = slice(tch * 512 + hh * 256,
                                       tch * 512 + (hh + 1) * 256)
                            (sdma if hh == 0 else adma)(
                                out=io["y"][:, sl],
                                in_=yo_c[:, hh * 256:(hh + 1) * 256])
                    else:
                        self_ln(tc, psum, small, ones_f, eps_t, z2, None, yo_c,
                                ln2g, ln2b, lean, f"ln2_{l}", tch)
                        sl = slice(tch * 512, (tch + 1) * 512)
                        dq(out=io["y"][:, sl], in_=yo_c)

            for step in range(TC):
                if step + 2 < TC:
                    self_ln(tc, psum, small, ones_f, eps_t, z1, yl1b, None,
                            ln1g, ln1b, lean, f"ln1_{l}", step + 2)
                emit_ffn(step)
                if step >= 1:
                    emit_ln2(step - 1)
            for f in pend_ffn:
                f()
            pend_ffn.clear()
            emit_ln2(TC - 1)


def self_ln(tc, psum, small, ones_f, eps_t, z, youtb, youtf, g_ap, b_ap,
            lean, nm, tch, col0=None, W=512, pst_ext=None):
    """LayerNorm chunk over the partition (feature) axis of z [P, TOKS].

    Stats via fp32r ones-matmul partition reduction into a paired psum
    (p1 = sum | p2 = sumsq); output written bf16 (youtb) or f32 (youtf).
    pst_ext supplies external (p1, p2) psum APs for drain-time chunks so
    they don't serialize on the shared single-buffered stats bank.
    """
    nc = tc.nc
    if col0 is None:
        col0 = tch * 512
    sl = slice(col0, col0 + W)
    sq = small.tile([P, 512], f32r, tag="sq", bufs=3, name=f"sq_{nm}_{tch}")
    nc.gpsimd.tensor_tensor(out=sq[:, 0:W], in0=z[:, sl], in1=z[:, sl],
                            op=ALU.mult)
    if pst_ext is None:
        pst = psum.tile([P, 1024], f32, tag="st", bufs=1,
                        name=f"lns_{nm}_{tch}")
        p1, p2 = pst[:, 0:W], pst[:, 512:512 + W]
    else:
        p1, p2 = pst_ext
    nc.tensor.matmul(p1, ones_f, z[:, sl], start=True, stop=True)
    nc.tensor.matmul(p2, ones_f, sq[:, 0:W], start=True, stop=True)
    eps2, lnp = eps_t
    musq = small.tile([P, 512], f32, tag="musq", bufs=2, name=f"mu_{nm}_{tch}")
    nc.scalar.activation(out=musq[:, 0:W], in_=p1, func=AF.Square)
    varq = small.tile([P, 512], f32, tag="varq", bufs=3, name=f"va_{nm}_{tch}")
    nc.vector.scalar_tensor_tensor(
        out=varq[:, 0:W], in0=p2, scalar=float(P), in1=musq[:, 0:W],
        op0=ALU.mult, op1=ALU.subtract)
    nc.scalar.activation(out=varq[:, 0:W], in_=varq[:, 0:W], func=AF.Ln,
                         bias=eps2)
    nc.scalar.activation(out=varq[:, 0:W], in_=varq[:, 0:W], func=AF.Exp,
                         scale=-0.5, bias=lnp)
    zc = small.tile([P, 512], f32, tag="zc", bufs=2, name=f"zc_{nm}_{tch}")
    nc.vector.scalar_tensor_tensor(
        out=zc[:, 0:W], in0=p1, scalar=-1.0 / P, in1=z[:, sl],
        op0=ALU.mult, op1=ALU.add)
    out = youtb if youtb is not None else youtf
    osl = out[:, sl] if out.shape[-1] == TOKS else out[:, col0 - tch * 512:
                                                      col0 - tch * 512 + W]
    if lean:
        # g == 1, b == 0: out = zc * rstd as a plain Pool tensor-tensor
        nc.gpsimd.tensor_tensor(out=osl, in0=zc[:, 0:W], in1=varq[:, 0:W],
                                op=ALU.mult)
    else:
        nc.vector.scalar_tensor_tensor(
            out=zc[:, 0:W], in0=zc[:, 0:W], scalar=g_ap, in1=varq[:, 0:W],
            op0=ALU.mult, op1=ALU.mult)
        nc.vector.tensor_scalar(out=osl, in0=zc[:, 0:W], scalar1=b_ap,
                                scalar2=None, op0=ALU.add)


_CACHE = {}


def _build(lean=True):
    key = ("nc", lean)
    if key in _CACHE:
        return _CACHE[key]
    nc = bacc.Bacc("TRN2", target_bir_lowering=False, debug=False, num_devices=8)
    io = {}

    def inp(name, shape, dt):
        io[name] = nc.dram_tensor(name, shape, dt, kind="ExternalInput").ap()

    inp("x_nm", [P, TL, NB, H], bf16)
    inp("s_t", [NB, P, N], bf16)
    inp("pe_t", [P, TL], f32)
    inp("wb", [P, BCOLS], bf16)
    inp("wf", [P, FCOLS], f32)
    inp("ones_f", [P, P], f32r)
    inp("bvr", [1, L * H], bf16)
    io["y"] = nc.dram_tensor("y", [P, TOKS], f32, kind="ExternalOutput").ap()

    with tile.TileContext(nc) as t:
        _emit(t, io, lean)
    nc.compile()
    _CACHE[key] = (nc, list(io))
    return nc, list(io)


def _host_prep(inputs):
    """Build the shared (replicated) device arrays from the full inputs."""
    x = np.asarray(inputs["x"], np.float32)
    x = (x @ np.asarray(inputs["conv_W"], np.float32)[0]) * SQH
    edge = np.asarray(inputs["edge_index"])

    src = np.concatenate([edge[0], np.arange(N, dtype=edge.dtype)])
    dst = np.concatenate([edge[1], np.arange(N, dtype=edge.dtype)])
    deg = np.zeros(N, np.float32)
    np.add.at(deg, dst, 1.0)
    dinv = 1.0 / np.sqrt(deg)
    normv = (dinv[src] * dinv[dst]).astype(np.float32)
    S = np.zeros((N, N), np.float32)
    np.add.at(S, (dst, src), normv)
    s_t = np.ascontiguousarray(S.T.reshape(NB, P, N)).astype(bfnp)

    pos = np.arange(T, dtype=np.float32)[:, None]
    ii = np.arange(0, H, 2, dtype=np.float32)
    pes = np.sin(pos / (10000.0 ** (2.0 * ii / H))).astype(np.float32)
    pec = np.cos(pos / (10000.0 ** (2.0 * (ii + 1.0) / H))).astype(np.float32)
    pe = np.stack([pes, pec], axis=-1).reshape(T, H).astype(np.float32)

    conv_W = np.asarray(inputs["conv_W"], np.float32)
    Wqkv = np.asarray(inputs["Wqkv"], np.float32)
    Wo = np.asarray(inputs["Wo"], np.float32)
    W1 = np.asarray(inputs["W1"], np.float32)
    W2 = np.asarray(inputs["W2"], np.float32)
    bqkv = np.asarray(inputs["bqkv"], np.float32).reshape(L, 3, P)
    conv_b = np.asarray(inputs["conv_b"], np.float32)
    b1v = np.asarray(inputs["b1"], np.float32).reshape(L, NF, P)

    wbp = np.zeros((P, BCOLS), np.float32)
    wfp = np.zeros((P, FCOLS), np.float32)
    for l in range(L):
        b = _boffs(l)
        f = _foffs(l)
        wbp[:, b["convW"]:b["convW"] + H] = conv_W[l]
        wbp[:, b["wqkvT"]:b["wqkvT"] + 3 * H] = Wqkv[l].T
        wbp[:, b["woT"]:b["woT"] + H] = Wo[l].T
        wbp[:, b["w1T"]:b["w1T"] + DFF] = W1[l].T
        wbp[:, b["w2T"]:b["w2T"] + NF * H] = (
            W2[l].T.reshape(NF, P, H).transpose(1, 0, 2).reshape(P, NF * H))
        wfp[:, f["convb_s"]] = conv_b[l] * SQH
        wfp[:, f["bq"]] = bqkv[l, 0]
        wfp[:, f["bk"]] = bqkv[l, 1]
        wfp[:, f["bo"]] = np.asarray(inputs["bo"], np.float32)[l]
        wfp[:, f["b1"]:f["b1"] + NF] = b1v[l].T
        wfp[:, f["b2"]] = np.asarray(inputs["b2"], np.float32)[l]
        wfp[:, f["ln1g"]] = np.asarray(inputs["ln1_g"], np.float32)[l]
        wfp[:, f["ln1b"]] = np.asarray(inputs["ln1_b"], np.float32)[l]
        wfp[:, f["ln2g"]] = np.asarray(inputs["ln2_g"], np.float32)[l]
        wfp[:, f["ln2b"]] = np.asarray(inputs["ln2_b"], np.float32)[l]
    wfp[:, L * _LF] = EPS
    wfp[:, L * _LF + 1] = P * P * EPS
    wfp[:, L * _LF + 2] = math.log(P)
    o = L * _LW
    wbp[:, o:o + P] = 1.0                                   # ones_b
    wbp[:, o + P:o + 2 * P] = -NEG_BIG * np.eye(P)          # negI
    wbp[:, o + 2 * P:o + 3 * P] = (                         # slt[k, v] = v < k
        np.arange(P)[None, :] < np.arange(P)[:, None]).astype(np.float32)
    wbp[:, o + 3 * P:o + 4 * P] = -NEG_BIG                  # negrow (rank-1 kill)
    # v-bias rows (one [1,H] row per layer for the rank-1 bias matmul)
    bvr = bqkv[:, 2, :].reshape(1, L * H)

    lean = all(
        not np.asarray(inputs[k], np.float32).any()
        for k in ("conv_b", "bqkv", "bo", "b1", "b2", "ln1_b", "ln2_b")
    ) and all(
        np.all(np.asarray(inputs[k], np.float32) == 1.0)
        for k in ("ln1_g", "ln2_g"))

    shared = {
        "s_t": s_t,
        "wb": wbp.astype(bfnp),
        "wf": wfp,
        "ones_f": np.ones((P, P), np.float32),
    }
    return shared, x, pe, bvr.astype(bfnp), lean


def make_in_maps(inputs):
    shared, x, pe, bvr, lean = _host_prep(inputs)
    shared["bvr"] = bvr
    in_maps = []
    for core in range(8):
        t0 = core * TL
        m = dict(shared)
        m["x_nm"] = np.ascontiguousarray(
            x[:, t0:t0 + TL, :].reshape(NB, P, TL, H).transpose(1, 2, 0, 3)
        ).astype(bfnp)
        m["pe_t"] = np.ascontiguousarray(pe[t0:t0 + TL].T)
        in_maps.append(m)
    return in_maps, lean


def kernel(**inputs):
    in_maps, lean = make_in_maps(inputs)
    nc, _ = _build(lean)
    res = run_bass_kernel_spmd(nc, in_maps, list(range(8)))

    out = np.zeros((N, T, H), np.float32)
    for core in range(8):
        t0 = core * TL
        yf = res.results[core]["y"].reshape(P, TL, N)
        for t in range(TL):
            out[:, t0 + t, :] = yf[:, t, :].T
    return out



# revision 90
# speedup vs baseline: 5.1319x; 5.1319x over previous
"""Trainium2 Bass kernel for nn_GCNTime (GCN + per-t causal transformer over nodes).

Sharding: T=16 time steps across 8 cores (2 per core). The graph (dense
normalized adjacency) is replicated; every stage is independent across t,
so there are no collectives.

v4 layout notes (per core):
- Activations are feature-major [128 partitions, token] (token = t*2048+node).
- PSUM is managed as three tag pools: "big"/"st" [128,1024] (two banks) and
  "pA"/"pB" [128,512]; paired 1024-wide tiles halve evac/exp instruction
  counts.
- Attention scores are computed transposed (keys on partitions); the causal
  block-triangle is trimmed at 128-column granularity and only the single
  diagonal 128x128 sub-block per key-block needs a mask multiply.
- Softmax denominator and LN statistics are partition reductions via
  ones-matmuls.
- LayerNorm rstd = exp(-0.5*ln(P^2*var + P^2*eps) + ln P): Ln/Exp share the
  one Act LUT set (natural_log_exp_and_others, id 6) with attention's Exp
  and all Relu/Square/Identity/Copy ops; the set is pre-loaded once so the
  Act engine never reloads activation tables mid-kernel.
- Layer-1 GCN aggregation runs as two half-sweeps over the resident s
  tiles with the psum evac / conv / qkv chains interleaved into the second
  half, hiding those chains behind PE matmul streams.
- FFN f1 evacs alternate Act/DVE evenly (4/4) and the f1 ring is 4 deep so
  the w2 accumulation never waits on a relu evac.
- Cross-boundary tail deferral: each attention chunk's last two dnav
  groups are re-emitted behind the NEXT chunk's first score pair, and each
  FFN step's last two w2 groups + z2 residual ride behind the next step's
  first w1 pairs, so the PE never stalls waiting for a boundary's final
  exp/relu evacuations (in-order queues would otherwise serialize them).
- Layer0 -> layer1 node-major input is produced by DMA XBAR transposes
  (bf16), not PE transposes. The final output stays feature-major and is
  transposed on the host.
- Weights arrive as two packed DMAs (bf16 + f32); only the conv/qkv/wo
  columns + attention masks ride the startup-critical s_t stream -- the
  1MB of FFN weight columns load after the layer-0 aggregation on the
  then-idle scalar DMA queue.
- Attention epilogues (softmax reciprocal -> wo matmul -> z1 residual)
  are deferred two chunks so their cross-engine chains never block the
  next chunks' score matmuls.
- Engine placement tuned end-to-end: post-relu +pe on DVE, agg psum
  evacs split DVE/Act to match consumer timing, FFN relu evacs 4/4
  Act/DVE, LN sq/varq tile rings 3 deep.
"""

import math
from contextlib import ExitStack

import numpy as np
import ml_dtypes

import concourse.bacc as bacc
import concourse.tile as tile
from concourse import mybir
from concourse.bass_utils import run_bass_kernel_spmd

P = 128
N = 2048          # nodes
T = 16            # total time steps
TL = 2            # time steps per core
NB = N // P       # node blocks (16)
H = 128
DFF = 2048
NF = DFF // P     # ffn chunks (16)
L = 2
TOKS = TL * N     # tokens per core (4096)
NDC = N // 512    # 512-wide node chunks (4)
TC = TOKS // 512  # 512-wide token chunks (8)
EPS = 1e-5
SCALE = 1.0 / math.sqrt(H)
SQH = math.sqrt(H)

f32 = mybir.dt.float32
f32r = mybir.dt.float32r
bf16 = mybir.dt.bfloat16
bfnp = ml_dtypes.bfloat16

AF = mybir.ActivationFunctionType
ALU = mybir.AluOpType

# ---- packed weight column offsets (bf16 pack) ----
# per layer: convW H | wqkvT 3H | woT H | w1T DFF | w2T NF*H
_LW = H + 3 * H + H + DFF + NF * H          # 4736
BCOLS = L * _LW + 4 * P                      # + ones_b + negI + slt + negrow
NEG_BIG = float(2 ** 20)
# f32 pack per layer: convb_s 1 | bq 1 | bk 1 | bo 1 | b1 NF | b2 1 |
#                     ln1g 1 | ln1b 1 | ln2g 1 | ln2b 1
_LF = 8 + NF
FCOLS = L * _LF + 3                          # + eps, P^2*eps, ln(P)


def _boffs(l):
    o = l * _LW
    return {
        "convW": o, "wqkvT": o + H, "woT": o + 4 * H,
        "w1T": o + 5 * H, "w2T": o + 5 * H + DFF,
    }


def _foffs(l):
    o = l * _LF
    return {
        "convb_s": o, "bq": o + 1, "bk": o + 2, "bo": o + 3,
        "b1": o + 4, "b2": o + 4 + NF, "ln1g": o + 5 + NF,
        "ln1b": o + 6 + NF, "ln2g": o + 7 + NF, "ln2b": o + 8 + NF,
    }


def _emit(tc, io, lean):
    nc = tc.nc
    with ExitStack() as ctx:
        consts = ctx.enter_context(tc.tile_pool(name="consts", bufs=1))
        spool = ctx.enter_context(tc.tile_pool(name="spool", bufs=14))
        xpool = ctx.enter_context(tc.tile_pool(name="xpool", bufs=1))
        resid = ctx.enter_context(tc.tile_pool(name="resid", bufs=2))
        hbp = ctx.enter_context(tc.tile_pool(name="hbp", bufs=2))
        qkvp = ctx.enter_context(tc.tile_pool(name="qkvp", bufs=2))
        apool = ctx.enter_context(tc.tile_pool(name="apool", bufs=3))
        ffp = ctx.enter_context(tc.tile_pool(name="ffp", bufs=4))
        small = ctx.enter_context(tc.tile_pool(name="small", bufs=2))
        psum = ctx.enter_context(tc.tile_pool(name="psum", bufs=2, space="PSUM"))

        nc.scalar.add_instruction(mybir.InstLoadActFuncSet(
            name=nc.get_next_instruction_name(), act_func_set_id=6))

        sdma = nc.sync.dma_start
        adma = nc.scalar.dma_start
        wdma = nc.gpsimd.dma_start

        # ---- initial DMAs: critical data first (xnm t-halves + s_t c=0,1),
        # then packed weights on the software-DGE queue ----
        xnm = xpool.tile([P, TL, NB, P], bf16, tag="xnm", name="xnm0")
        sdma(out=xnm[:, 0, 0:4], in_=io["x_nm"][:, 0, 0:4])
        s_pre = []
        s0 = spool.tile([P, N], bf16, tag="s_tile", name="s0_0")
        adma(out=s0[:, 0:1024], in_=io["s_t"][0][:, 0:1024])
        sdma(out=xnm[:, 0, 4:NB], in_=io["x_nm"][:, 0, 4:NB])
        adma(out=s0[:, 1024:2048], in_=io["s_t"][0][:, 1024:2048])
        s_pre.append(s0)
        s1 = spool.tile([P, N], bf16, tag="s_tile", name="s0_1")
        adma(out=s1, in_=io["s_t"][1])
        sdma(out=xnm[:, 1], in_=io["x_nm"][:, 1])
        s_pre.append(s1)

        # wb pieces are DMA'd interleaved with the layer-0 s_t stream (below)
        # so the big weight transfers don't block the startup-critical loads
        wb = consts.tile([P, BCOLS], bf16, tag="wb")
        # layer-0 pieces + masks stream during the startup-critical window;
        # layer-1 pieces wait until the DMA engines go idle (conv/qkv phase)
        wb_pieces = [
            (0, 5 * H), (L * _LW, 4 * P),
        ]
        wb_late = [
            (5 * H, DFF), (5 * H + DFF, NF * H),
            (_LW, 5 * H), (_LW + 5 * H, DFF), (_LW + 5 * H + DFF, NF * H),
        ]
        wf = consts.tile([P, FCOLS], f32, tag="wf")
        wdma(out=wf, in_=io["wf"])
        pe_t = consts.tile([P, TL], f32, tag="pe_t")
        wdma(out=pe_t, in_=io["pe_t"])
        ones_f = consts.tile([P, P], f32r, tag="ones_f")
        wdma(out=ones_f, in_=io["ones_f"])
        bvr = consts.tile([1, L * H], bf16, tag="bvr")
        wdma(out=bvr, in_=io["bvr"])

        ones_b = wb[:, L * _LW:L * _LW + P]
        negI = wb[:, L * _LW + P:L * _LW + 2 * P]
        slt = wb[:, L * _LW + 2 * P:L * _LW + 3 * P]
        negrow = wb[:, L * _LW + 3 * P:L * _LW + 4 * P]
        eps_t = (wf[:, L * _LF + 1:L * _LF + 2], wf[:, L * _LF + 2:L * _LF + 3])

        s_next = None
        for l in range(L):
            bo_ = _boffs(l)
            fo = _foffs(l)
            convW = wb[:, bo_["convW"]:bo_["convW"] + H]
            wqkvT = wb[:, bo_["wqkvT"]:bo_["wqkvT"] + 3 * H]
            woT = wb[:, bo_["woT"]:bo_["woT"] + H]
            w1T = wb[:, bo_["w1T"]:bo_["w1T"] + DFF]
            w2T = wb[:, bo_["w2T"]:bo_["w2T"] + NF * H]
            convb_s = wf[:, fo["convb_s"]:fo["convb_s"] + 1]
            bq = wf[:, fo["bq"]:fo["bq"] + 1]
            bk = wf[:, fo["bk"]:fo["bk"] + 1]
            bo_ap = wf[:, fo["bo"]:fo["bo"] + 1]
            b1 = wf[:, fo["b1"]:fo["b1"] + NF]
            b2 = wf[:, fo["b2"]:fo["b2"] + 1]
            ln1g = wf[:, fo["ln1g"]:fo["ln1g"] + 1]
            ln1b = wf[:, fo["ln1b"]:fo["ln1b"] + 1]
            ln2g = wf[:, fo["ln2g"]:fo["ln2g"] + 1]
            ln2b = wf[:, fo["ln2b"]:fo["ln2b"] + 1]

            # ======== GCN aggregation: agg[f, tok] = sum_s x[s,f] * S^T[s, dst]
            # 5 accumulator psums covering [t, dc]: big(t0,dc01) big(t0,dc23)
            # st(t1,dc01) ps(t1,dc2) ps(t1,dc3)
            agA = psum.tile([P, 1024], f32, tag="big", name=f"agA{l}")
            agB = psum.tile([P, 1024], f32, tag="big", name=f"agB{l}")
            agC = psum.tile([P, 1024], f32, tag="st", bufs=1, name=f"agC{l}")
            agD = psum.tile([P, 512], f32, tag="pA", bufs=1, name=f"agD{l}")
            agE = psum.tile([P, 512], f32, tag="pB", bufs=1, name=f"agE{l}")
            dsts = [
                (0, agA[:, 0:512]), (0, agA[:, 512:1024]),
                (0, agB[:, 0:512]), (0, agB[:, 512:1024]),
                (1, agC[:, 0:512]), (1, agC[:, 512:1024]),
                (1, agD), (1, agE),
            ]
            aggb = (hbp.tile([P, TOKS], bf16, tag="hb", name=f"aggb{l}")
                    if l > 0 else None)
            h1b = hbp.tile([P, TOKS], bf16, tag="hb", name=f"h1b{l}")
            evacs = [
                lambda: nc.vector.tensor_copy(aggb[:, 0:1024], agA),
                lambda: nc.scalar.copy(aggb[:, 1024:2048], agB),
                lambda: nc.vector.tensor_copy(aggb[:, 2048:3072], agC),
                lambda: nc.scalar.copy(aggb[:, 3072:3584], agD),
                lambda: nc.vector.tensor_copy(aggb[:, 3584:4096], agE),
            ]

            def agg_sweep(idxs, c0, c1):
                for c in range(c0, c1):
                    s_tile = s_next[c]
                    for i in idxs:
                        t, dst = dsts[i]
                        dc = i % 4
                        nc.tensor.matmul(
                            dst, xnm[:, t, c],
                            s_tile[:, dc * 512:(dc + 1) * 512],
                            start=(c == 0), stop=(c == NB - 1))

            # GCN linear -> relu*sqrt(H) -> +pe -> h1b (bf16 residual)
            def emit_conv(pr):
                t = (2 * pr) // NDC
                pc = psum.tile([P, 1024], f32, tag="big", name=f"conv{l}_{pr}")
                sl = slice(pr * 1024, (pr + 1) * 1024)
                nc.tensor.matmul(pc[:, 0:512], convW,
                                 aggb[:, pr * 1024:pr * 1024 + 512],
                                 start=True, stop=True)
                nc.tensor.matmul(pc[:, 512:1024], convW,
                                 aggb[:, pr * 1024 + 512:(pr + 1) * 1024],
                                 start=True, stop=True)
                nc.scalar.activation(out=h1b[:, sl], in_=pc, func=AF.Relu,
                                     scale=SQH, bias=convb_s)
                nc.vector.tensor_scalar(
                    out=h1b[:, sl], in0=h1b[:, sl], scalar1=pe_t[:, t:t + 1],
                    scalar2=None, op0=ALU.add)

            # qkv for one t (q,k feature-major; v node-major)
            qs, ks, vns, attns = [], [], [], []

            def emit_qkv(t):
                q_t = qkvp.tile([P, N], bf16, tag="q_t", name=f"q{l}_{t}")
                k_t = qkvp.tile([P, N], bf16, tag="k_t", name=f"k{l}_{t}")
                for part, dest, bias in ((0, q_t, bq), (1, k_t, bk)):
                    for hp in range(2):
                        pq = psum.tile([P, 1024], f32, tag="big",
                                       name=f"qk{l}_{t}_{part}_{hp}")
                        for hh in range(2):
                            ncc = hp * 2 + hh
                            nc.tensor.matmul(
                                pq[:, hh * 512:(hh + 1) * 512],
                                wqkvT[:, part * H:(part + 1) * H],
                                h1b[:, t * N + ncc * 512:t * N + (ncc + 1) * 512],
                                start=True, stop=True)
                        osl = dest[:, hp * 1024:(hp + 1) * 1024]
                        if part == 0:
                            nc.vector.tensor_scalar(
                                out=osl, in0=pq, scalar1=bias,
                                scalar2=None, op0=ALU.add)
                        else:
                            nc.scalar.activation(
                                out=osl, in_=pq, func=AF.Identity, bias=bias)
                vn = qkvp.tile([P, NB, P], bf16, tag="vn", name=f"vn{l}_{t}")
                for hp in range(2):
                    pv = psum.tile([P, 1024], f32, tag="big",
                                   name=f"v{l}_{t}_{hp}")
                    for j in range(8):
                        jj = hp * 8 + j
                        reg = pv[:, j * P:(j + 1) * P]
                        nc.tensor.matmul(
                            reg,
                            h1b[:, t * N + jj * P:t * N + (jj + 1) * P],
                            wqkvT[:, 2 * H:3 * H],
                            start=True, stop=lean)
                        if not lean:
                            # bias via rank-1 accumulation (ones row x bias row)
                            nc.tensor.matmul(
                                reg, ones_b[0:1, :],
                                bvr[:, l * H:(l + 1) * H],
                                start=False, stop=True)
                    nc.vector.tensor_copy(
                        vn[:, hp * 8:(hp + 1) * 8, :], pv)
                attnb = qkvp.tile([P, N], bf16, tag="attnb", name=f"at{l}_{t}")
                qs.append(q_t); ks.append(k_t); vns.append(vn); attns.append(attnb)

            if l == 0:
                # layer 0: s_t streams from HBM; tile-major sweep keeps the
                # PE fed at the DMA arrival rate
                for c in range(NB):
                    if c < 2:
                        s_tile = s_pre[c]
                    else:
                        s_tile = spool.tile([P, N], bf16, tag="s_tile",
                                            name=f"s{l}_{c}")
                        (sdma if c % 2 == 0 else adma)(
                            out=s_tile, in_=io["s_t"][c])
                    if 1 <= c <= len(wb_pieces):
                        off, w = wb_pieces[c - 1]
                        (sdma if c % 2 == 1 else adma)(
                            out=wb[:, off:off + w], in_=io["wb"][:, off:off + w])
                    for i, (t, dst) in enumerate(dsts):
                        dc = i % 4
                        nc.tensor.matmul(
                            dst, xnm[:, t, c], s_tile[:, dc * 512:(dc + 1) * 512],
                            start=(c == 0), stop=(c == NB - 1))
                # prefetch next layer's adjacency tiles during this layer's
                # compute (DMA engines are otherwise idle mid-layer)
                s_next = []
                for c in range(NB):
                    st_ = spool.tile([P, N], bf16, tag="s_tile",
                                     name=f"s{l + 1}_{c}")
                    sdma(out=st_, in_=io["s_t"][c])
                    s_next.append(st_)
                for off, w in wb_late:
                    sdma(out=wb[:, off:off + w], in_=io["wb"][:, off:off + w])
                for e in evacs:
                    e()
                for pr in range(TC // 2):
                    emit_conv(pr)
                emit_qkv(0)
                emit_qkv(1)
            else:
                # layer 1: s tiles prefetched. Sweep tiles 0..7 for all
                # psums first (tile release order keeps the 13-deep spool
                # ring from gating the tail prefetch DMAs), then finish each
                # psum over tiles 8..15 with its evac/conv/qkv interleaved so
                # those chains hide behind the next psum's matmuls.
                for idxs in ([0, 1], [2, 3], [4, 5], [6, 7]):
                    agg_sweep(idxs, 0, NB // 2)
                agg_sweep([0, 1], NB // 2, NB)
                evacs[0]()
                emit_conv(0)
                agg_sweep([2, 3], NB // 2, NB)
                evacs[1]()
                emit_conv(1)
                emit_qkv(0)
                agg_sweep([4, 5], NB // 2, NB)
                evacs[2]()
                emit_conv(2)
                agg_sweep([6, 7], NB // 2, NB)
                evacs[3]()
                evacs[4]()
                emit_conv(3)
                emit_qkv(1)

            # ======== attention, (ic, t)-interleaved, causal-trimmed
            z1 = resid.tile([P, TOKS], f32r, tag="resid", name=f"z1_{l}")
            yl1b = hbp.tile([P, TOKS], bf16, tag="hb", name=f"yl1b{l}")
            ci = 0
            pend_ep = []
            pend_dnav = []

            def attn_chunk(ic, t):
                    nonlocal ci
                    q_t, k_t, vn, attnb = qs[t], ks[t], vns[t], attns[t]
                    jmax = 4 * ic + 4
                    npair = jmax // 2
                    q0 = ic * 512

                    def rs(j):
                        return 128 * max(0, j - 4 * ic)

                    # pd/pv double-buffer across chunks: even chunks use the
                    # pA/pB half-banks, odd chunks the st pair
                    if ci % 2 == 0:
                        pd = psum.tile([P, 512], f32, tag="pA", bufs=1,
                                       name=f"pd{l}_{t}_{ic}")
                        pv = psum.tile([P, 512], f32, tag="pB", bufs=1,
                                       name=f"pv{l}_{t}_{ic}")
                    else:
                        pdpv = psum.tile([P, 1024], f32, tag="st", bufs=1,
                                         name=f"pdpv{l}_{t}_{ic}")
                        pd, pv = pdpv[:, 0:512], pdpv[:, 512:1024]
                    pairs = []

                    def emit_dnav(p):
                        A_p, j0 = pairs[p]
                        for h in range(2):
                            j = j0 + h
                            r0 = rs(j)
                            nc.tensor.matmul(
                                pd[:, r0:512], ones_b,
                                A_p[:, h * 512 + r0:(h + 1) * 512],
                                start=(j == 0), stop=(j == jmax - 1))
                        for h in range(2):
                            j = j0 + h
                            r0 = rs(j)
                            nc.tensor.matmul(
                                pv[:, r0:512], vn[:, j, :],
                                A_p[:, h * 512 + r0:(h + 1) * 512],
                                start=(j == 0), stop=(j == jmax - 1))

                    for p in range(npair):
                        j0 = 2 * p
                        pa = psum.tile([P, 1024], f32, tag="big",
                                       name=f"sc{l}_{t}_{ic}_{p}")
                        diag = j0 >= 4 * ic
                        for h in range(2):
                            j = j0 + h
                            r0 = rs(j)
                            nc.tensor.matmul(
                                pa[:, h * 512 + r0:(h + 1) * 512],
                                k_t[:, j * P:(j + 1) * P],
                                q_t[:, q0 + r0:q0 + 512],
                                start=True, stop=not diag,
                                skip_group_check=diag)
                            if diag:
                                # causal mask: add -BIG to the upper triangle
                                # of the diagonal 128x128 sub-block (exp -> 0)
                                nc.tensor.matmul(
                                    pa[:, h * 512 + r0:h * 512 + r0 + 128],
                                    negI, slt, start=False, stop=True,
                                    skip_group_check=True)
                        A_p = apool.tile([P, 1024], bf16, tag="A",
                                         name=f"A{l}_{t}_{ic}_{p}")
                        r0 = rs(j0)
                        nc.scalar.activation(out=A_p[:, r0:1024],
                                             in_=pa[:, r0:1024],
                                             func=AF.Exp, scale=SCALE)
                        pairs.append((A_p, j0))
                        if p == 0 and pend_dnav:
                            # previous chunk's tail dnav groups ride behind
                            # our first scores: their exp inputs are ready
                            # by now, so the PE never stalls on the exp of
                            # a chunk's last pairs
                            for f in pend_dnav:
                                f()
                            pend_dnav.clear()
                        if p >= 2:
                            emit_dnav(p - 2)
                    pend_dnav.append(
                        lambda p=npair - 2, f=emit_dnav: f(p))
                    pend_dnav.append(
                        lambda p=npair - 1, f=emit_dnav: f(p))

                    # defer the chunk epilogue (rec/attnb/po/z1) until the
                    # next chunk's scores are on the PE queue, so the po
                    # matmul's wait on the DVE chain doesn't block them
                    def mk_epilogue(pd=pd, pv=pv, t=t, ic=ic, ci=ci,
                                    attnb=attnb, q0=q0):
                        def ep():
                            rec = small.tile([P, 512], f32, tag="rec", bufs=2,
                                             name=f"rec{l}_{t}_{ic}")
                            nc.vector.reciprocal(rec, pd)
                            nc.vector.tensor_tensor(
                                out=attnb[:, q0:q0 + 512], in0=pv,
                                in1=rec, op=ALU.mult)
                            po = psum.tile([P, 512], f32,
                                           tag=("pA" if ci % 2 == 0 else "pB"),
                                           bufs=1, name=f"wo{l}_{t}_{ic}")
                            nc.tensor.matmul(po, woT, attnb[:, q0:q0 + 512],
                                             start=True, stop=True)
                            sl = slice(t * N + q0, t * N + q0 + 512)
                            nc.vector.scalar_tensor_tensor(
                                out=z1[:, sl], in0=po, scalar=bo_ap,
                                in1=h1b[:, sl], op0=ALU.add, op1=ALU.add)
                        return ep
                    pend_ep.append(mk_epilogue())
                    ci += 1
                    if ci == 4:
                        # hoist the first two LN1 chunks (their z1 tokens are
                        # complete) so the LN chain overlaps late attention
                        for f in pend_dnav:
                            f()
                        pend_dnav.clear()
                        while pend_ep:
                            pend_ep.pop(0)()
                        for e in range(2):
                            self_ln(tc, psum, small, ones_f, eps_t, z1, yl1b,
                                    None, ln1g, ln1b, lean, f"ln1_{l}", e)
                    elif len(pend_ep) > 2:
                        pend_ep.pop(0)()

            # agg/conv/qkv, with attention chunk (0,0) emitted before
            # qkv(t=1) so its scores fill the qkv evac bubbles on the PE
            emit_front()
            for ic in range(NDC):
                for t in range(TL):
                    attn_chunk(ic, t)
            for f in pend_dnav:
                f()
            pend_dnav.clear()
            while pend_ep:
                pend_ep.pop(0)()

            # ======== FFN (+ remaining LN1) -> z2 ; then LN2
            z2 = resid.tile([P, TOKS], f32r, tag="resid", name=f"z2_{l}")

            pend_ffn = []

            def emit_ffn(tch):
                sl = slice(tch * 512, (tch + 1) * 512)
                p2 = psum.tile([P, 512], f32, tag="pA", bufs=1,
                               name=f"ff2{l}_{tch}")
                f1s = []

                def emit_w2(cp):
                    f1 = f1s[cp]
                    for h in range(2):
                        c = 2 * cp + h
                        nc.tensor.matmul(
                            p2, w2T[:, c * P:(c + 1) * P],
                            f1[:, h * 512:(h + 1) * 512],
                            start=(c == 0), stop=(c == NF - 1))

                for cp in range(NF // 2):
                    p1 = psum.tile([P, 1024], f32, tag="big",
                                   name=f"ff1{l}_{tch}_{cp}")
                    for h in range(2):
                        c = 2 * cp + h
                        nc.tensor.matmul(
                            p1[:, h * 512:(h + 1) * 512],
                            w1T[:, c * P:(c + 1) * P], yl1b[:, sl],
                            start=True, stop=True)
                    f1 = ffp.tile([P, 1024], bf16, tag="f1",
                                  name=f"f1_{l}_{tch}_{cp}")
                    eng = (0, 1, 0, 1, 0, 1, 0, 1)[cp % 8]
                    if lean:
                        if eng == 0:
                            nc.scalar.activation(out=f1, in_=p1, func=AF.Relu)
                        else:
                            nc.vector.tensor_scalar(
                                out=f1, in0=p1, scalar1=0.0, scalar2=None,
                                op0=ALU.max)
                    else:
                        for h in range(2):
                            c = 2 * cp + h
                            hs = slice(h * 512, (h + 1) * 512)
                            if eng == 0:
                                nc.scalar.activation(
                                    out=f1[:, hs], in_=p1[:, hs], func=AF.Relu,
                                    bias=b1[:, c:c + 1])
                            else:
                                nc.vector.tensor_scalar(
                                    out=f1[:, hs], in0=p1[:, hs],
                                    scalar1=b1[:, c:c + 1], scalar2=0.0,
                                    op0=ALU.add, op1=ALU.max)
                    f1s.append(f1)
                    if cp == 1 and pend_ffn:
                        # previous step's deferred tail (last two w2 groups
                        # + z2 residual) rides behind our first w1 pairs:
                        # its relu inputs are long done, so the PE never
                        # stalls on a step's last evacs. Flushing at cp==1
                        # keeps the 4-deep f1 ring's eviction waits backward.
                        for f in pend_ffn:
                            f()
                        pend_ffn.clear()
                    if cp >= 2:
                        emit_w2(cp - 2)

                def ffn_tail(emit_w2=emit_w2, p2=p2, sl=sl):
                    emit_w2(NF // 2 - 2)
                    emit_w2(NF // 2 - 1)
                    nc.vector.scalar_tensor_tensor(
                        out=z2[:, sl], in0=p2, scalar=b2, in1=yl1b[:, sl],
                        op0=ALU.add, op1=ALU.add)
                pend_ffn.append(ffn_tail)

            # LN1 / FFN / LN2 software-pipelined per 512-token chunk; LN2
            # feeds the next layer's node-major input (DMA transpose) or the
            # final feature-major output DMA.
            if l < L - 1:
                youtb = hbp.tile([P, TOKS], bf16, tag="hb", name=f"ynb{l}")
                xnm = xpool.tile([P, TL, NB, P], bf16, tag="xnm", name="xnm1")
                yo = None
            else:
                youtb = None

            def emit_ln2(tch):
                if youtb is not None:
                    self_ln(tc, psum, small, ones_f, eps_t, z2, youtb, None,
                            ln2g, ln2b, lean, f"ln2_{l}", tch)
                    t, ic = tch // NDC, tch % NDC
                    nc.sync.dma_start_transpose(
                        xnm[:, t, 4 * ic:4 * ic + 4, :],
                        youtb[:, tch * 512:(tch + 1) * 512])
                else:
                    yo_c = small.tile([P, 512], f32, tag="yoc", bufs=2,
                                      name=f"yoc{tch}")
                    dq = sdma if tch % 2 == 0 else adma
                    if tch == TC - 1:
                        # split the last chunk for a shorter drain chain,
                        # with stats in the (now free) pA/pB banks so the
                        # halves don't serialize on the shared st bank
                        for hh in range(2):
                            pp = psum.tile([P, 512], f32,
                                           tag=("pA" if hh == 0 else "pB"),
                                           bufs=1, name=f"lnd{l}_{hh}")
                            self_ln(tc, psum, small, ones_f, eps_t, z2, None,
                                    yo_c, ln2g, ln2b, lean, f"ln2_{l}_h{hh}",
                                    tch, col0=tch * 512 + hh * 256, W=256,
                                    pst_ext=(pp[:, 0:256], pp[:, 256:512]))
                            sl = slice(tch * 512 + hh * 256,
                                       tch * 512 + (hh + 1) * 256)
                            (sdma if hh == 0 else adma)(
                                out=io["y"][:, sl],
                                in_=yo_c[:, hh * 256:(hh + 1) * 256])
                    else:
                        self_ln(tc, psum, small, ones_f, eps_t, z2, None, yo_c,
                                ln2g, ln2b, lean, f"ln2_{l}", tch)
                        sl = slice(tch * 512, (tch + 1) * 512)
                        dq(out=io["y"][:, sl], in_=yo_c)

            for step in range(TC):
                if step + 2 < TC:
                    self_ln(tc, psum, small, ones_f, eps_t, z1, yl1b, None,
                            ln1g, ln1b, lean, f"ln1_{l}", step + 2)
                emit_ffn(step)
                if step >= 1:
                    emit_ln2(step - 1)
            for f in pend_ffn:
                f()
            pend_ffn.clear()
            emit_ln2(TC - 1)


def self_ln(tc, psum, small, ones_f, eps_t, z, youtb, youtf, g_ap, b_ap,
            lean, nm, tch, col0=None, W=512, pst_ext=None):
    """LayerNorm chunk over the partition (feature) axis of z [P, TOKS].

    Stats via fp32r ones-matmul partition reduction into a paired psum
    (p1 = sum | p2 = sumsq); output written bf16 (youtb) or f32 (youtf).
    pst_ext supplies external (p1, p2) psum APs for drain-time chunks so
    they don't serialize on the shared single-buffered stats bank.
    """
    nc = tc.nc
    if col0 is None:
        col0 = tch * 512
    sl = slice(col0, col0 + W)
    sq = small.tile([P, 512], f32r, tag="sq", bufs=3, name=f"sq_{nm}_{tch}")
    nc.gpsimd.tensor_tensor(out=sq[:, 0:W], in0=z[:, sl], in1=z[:, sl],
                            op=ALU.mult)
    if pst_ext is None:
        pst = psum.tile([P, 1024], f32, tag="st", bufs=1,
                        name=f"lns_{nm}_{tch}")
        p1, p2 = pst[:, 0:W], pst[:, 512:512 + W]
    else:
        p1, p2 = pst_ext
    nc.tensor.matmul(p1, ones_f, z[:, sl], start=True, stop=True)
    nc.tensor.matmul(p2, ones_f, sq[:, 0:W], start=True, stop=True)
    eps2, lnp = eps_t
    musq = small.tile([P, 512], f32, tag="musq", bufs=2, name=f"mu_{nm}_{tch}")
    nc.scalar.activation(out=musq[:, 0:W], in_=p1, func=AF.Square)
    varq = small.tile([P, 512], f32, tag="varq", bufs=3, name=f"va_{nm}_{tch}")
    nc.vector.scalar_tensor_tensor(
        out=varq[:, 0:W], in0=p2, scalar=float(P), in1=musq[:, 0:W],
        op0=ALU.mult, op1=ALU.subtract)
    nc.scalar.activation(out=varq[:, 0:W], in_=varq[:, 0:W], func=AF.Ln,
                         bias=eps2)
    nc.scalar.activation(out=varq[:, 0:W], in_=varq[:, 0:W], func=AF.Exp,
                         scale=-0.5, bias=lnp)
    zc = small.tile([P, 512], f32, tag="zc", bufs=2, name=f"zc_{nm}_{tch}")
    nc.vector.scalar_tensor_tensor(
        out=zc[:, 0:W], in0=p1, scalar=-1.0 / P, in1=z[:, sl],
        op0=ALU.mult, op1=ALU.add)
    out = youtb if youtb is not None else youtf
    osl = out[:, sl] if out.shape[-1] == TOKS else out[:, col0 - tch * 512:
                                                      col0 - tch * 512 + W]
    if lean:
        # g == 1, b == 0: out = zc * rstd as a plain Pool tensor-tensor
        nc.gpsimd.tensor_tensor(out=osl, in0=zc[:, 0:W], in1=varq[:, 0:W],
                                op=ALU.mult)
    else:
        nc.vector.scalar_tensor_tensor(
            out=zc[:, 0:W], in0=zc[:, 0:W], scalar=g_ap, in1=varq[:, 0:W],
            op0=ALU.mult, op1=ALU.mult)
        nc.vector.tensor_scalar(out=osl, in0=zc[:, 0:W], scalar1=b_ap,
                                scalar2=None, op0=ALU.add)


_CACHE = {}


def _build(lean=True):
    key = ("nc", lean)
    if key in _CACHE:
        return _CACHE[key]
    nc = bacc.Bacc("TRN2", target_bir_lowering=False, debug=False, num_devices=8)
    io = {}

    def inp(name, shape, dt):
        io[name] = nc.dram_tensor(name, shape, dt, kind="ExternalInput").ap()

    inp("x_nm", [P, TL, NB, H], bf16)
    inp("s_t", [NB, P, N], bf16)
    inp("pe_t", [P, TL], f32)
    inp("wb", [P, BCOLS], bf16)
    inp("wf", [P, FCOLS], f32)
    inp("ones_f", [P, P], f32r)
    inp("bvr", [1, L * H], bf16)
    io["y"] = nc.dram_tensor("y", [P, TOKS], f32, kind="ExternalOutput").ap()

    with tile.TileContext(nc) as t:
        _emit(t, io, lean)
    nc.compile()
    _CACHE[key] = (nc, list(io))
    return nc, list(io)


def _host_prep(inputs):
    """Build the shared (replicated) device arrays from the full inputs."""
    x = np.asarray(inputs["x"], np.float32)
    x = (x @ np.asarray(inputs["conv_W"], np.float32)[0]) * SQH
    edge = np.asarray(inputs["edge_index"])

    src = np.concatenate([edge[0], np.arange(N, dtype=edge.dtype)])
    dst = np.concatenate([edge[1], np.arange(N, dtype=edge.dtype)])
    deg = np.zeros(N, np.float32)
    np.add.at(deg, dst, 1.0)
    dinv = 1.0 / np.sqrt(deg)
    normv = (dinv[src] * dinv[dst]).astype(np.float32)
    S = np.zeros((N, N), np.float32)
    np.add.at(S, (dst, src), normv)
    s_t = np.ascontiguousarray(S.T.reshape(NB, P, N)).astype(bfnp)

    pos = np.arange(T, dtype=np.float32)[:, None]
    ii = np.arange(0, H, 2, dtype=np.float32)
    pes = np.sin(pos / (10000.0 ** (2.0 * ii / H))).astype(np.float32)
    pec = np.cos(pos / (10000.0 ** (2.0 * (ii + 1.0) / H))).astype(np.float32)
    pe = np.stack([pes, pec], axis=-1).reshape(T, H).astype(np.float32)

    conv_W = np.asarray(inputs["conv_W"], np.float32)
    Wqkv = np.asarray(inputs["Wqkv"], np.float32)
    Wo = np.asarray(inputs["Wo"], np.float32)
    W1 = np.asarray(inputs["W1"], np.float32)
    W2 = np.asarray(inputs["W2"], np.float32)
    bqkv = np.asarray(inputs["bqkv"], np.float32).reshape(L, 3, P)
    conv_b = np.asarray(inputs["conv_b"], np.float32)
    b1v = np.asarray(inputs["b1"], np.float32).reshape(L, NF, P)

    wbp = np.zeros((P, BCOLS), np.float32)
    wfp = np.zeros((P, FCOLS), np.float32)
    for l in range(L):
        b = _boffs(l)
        f = _foffs(l)
        wbp[:, b["convW"]:b["convW"] + H] = conv_W[l]
        wbp[:, b["wqkvT"]:b["wqkvT"] + 3 * H] = Wqkv[l].T
        wbp[:, b["woT"]:b["woT"] + H] = Wo[l].T
        wbp[:, b["w1T"]:b["w1T"] + DFF] = W1[l].T
        wbp[:, b["w2T"]:b["w2T"] + NF * H] = (
            W2[l].T.reshape(NF, P, H).transpose(1, 0, 2).reshape(P, NF * H))
        wfp[:, f["convb_s"]] = conv_b[l] * SQH
        wfp[:, f["bq"]] = bqkv[l, 0]
        wfp[:, f["bk"]] = bqkv[l, 1]
        wfp[:, f["bo"]] = np.asarray(inputs["bo"], np.float32)[l]
        wfp[:, f["b1"]:f["b1"] + NF] = b1v[l].T
        wfp[:, f["b2"]] = np.asarray(inputs["b2"], np.float32)[l]
        wfp[:, f["ln1g"]] = np.asarray(inputs["ln1_g"], np.float32)[l]
        wfp[:, f["ln1b"]] = np.asarray(inputs["ln1_b"], np.float32)[l]
        wfp[:, f["ln2g"]] = np.asarray(inputs["ln2_g"], np.float32)[l]
        wfp[:, f["ln2b"]] = np.asarray(inputs["ln2_b"], np.float32)[l]
    wfp[:, L * _LF] = EPS
    wfp[:, L * _LF + 1] = P * P * EPS
    wfp[:, L * _LF + 2] = math.log(P)
    o = L * _LW
    wbp[:, o:o + P] = 1.0                                   # ones_b
    wbp[:, o + P:o + 2 * P] = -NEG_BIG * np.eye(P)          # negI
    wbp[:, o + 2 * P:o + 3 * P] = (                         # slt[k, v] = v < k
        np.arange(P)[None, :] < np.arange(P)[:, None]).astype(np.float32)
    wbp[:, o + 3 * P:o + 4 * P] = -NEG_BIG                  # negrow (rank-1 kill)
    # v-bias rows (one [1,H] row per layer for the rank-1 bias matmul)
    bvr = bqkv[:, 2, :].reshape(1, L * H)

    lean = all(
        not np.asarray(inputs[k], np.float32).any()
        for k in ("conv_b", "bqkv", "bo", "b1", "b2", "ln1_b", "ln2_b")
    ) and all(
        np.all(np.asarray(inputs[k], np.float32) == 1.0)
        for k in ("ln1_g", "ln2_g"))

    shared = {
        "s_t": s_t,
        "wb": wbp.astype(bfnp),
        "wf": wfp,
        "ones_f": np.ones((P, P), np.float32),
    }
    return shared, x, pe, bvr.astype(bfnp), lean


def make_in_maps(inputs):
    shared, x, pe, bvr, lean = _host_prep(inputs)
    shared["bvr"] = bvr
    in_maps = []
    for core in range(8):
        t0 = core * TL
        m = dict(shared)
        m["x_nm"] = np.ascontiguousarray(
            x[:, t0:t0 + TL, :].reshape(NB, P, TL, H).transpose(1, 2, 0, 3)
        ).astype(bfnp)
        m["pe_t"] = np.ascontiguousarray(pe[t0:t0 + TL].T)
        in_maps.append(m)
    return in_maps, lean


def kernel(**inputs):
    in_maps, lean = make_in_maps(inputs)
    nc, _ = _build(lean)
    res = run_bass_kernel_spmd(nc, in_maps, list(range(8)))

    out = np.zeros((N, T, H), np.float32)
    for core in range(8):
        t0 = core * TL
        yf = res.results[core]["y"].reshape(P, TL, N)
        for t in range(TL):
            out[:, t0 + t, :] = yf[:, t, :].T
    return out

